# revision 36
# baseline (speedup 1.0000x reference)
"""GCN (7-layer, PyG-style symmetric-normalized message passing) on 8 TRN2
NeuronCores via Bass/Tile.

Strategy (graph/data parallel over nodes):
  - Nodes are assigned to 8 cores x 49 blocks of 128 slots each (load-balanced
    by in-degree so per-block message counts are nearly equal).
  - Per layer, per core:
      stage A: Z'' = dis * (h @ W) for the core's 6272 node slots (PE matmul
               per 128-node block + per-partition scale), node-major in SBUF.
      AllGather: bf16 Z'' shards -> full 50176-row table in local HBM.
      stage B: per dst block, gather Z''[src] rows for the block's edges
               (SWDGE dma_gather, int16 indices, table split in two halves to
               fit int16 range), build one-hot selector S on DVE
               (S[msg, slot] = (iota == segid)), and segment-sum on PE:
               O[feat, slot] += M_chunk.T @ S_chunk, with the self-loop chunk
               done as zbuf_block.T @ I directly from SBUF.
      epilogue: h' = relu(O * dis[dst] + b)  (DVE mult + ACT relu w/ bias).
  - Final: out = lin_w.T @ h7 + lin_b, one row per core, host reassembles.

All index/normalization prep is host-side numpy (graph routing), baked into
per-core input tensors; the float pipeline runs on device.
"""

import math
import os
import sys
from dataclasses import dataclass

import numpy as np

sys.path.insert(0, "/opt/trn_rl_repo")

import ml_dtypes  # noqa: E402

BF16 = ml_dtypes.bfloat16


@dataclass
class GCNConfig:
    n_nodes: int = 50000
    n_edges: int = 600000
    feat: int = 128
    n_layers: int = 7
    n_cores: int = 8
    half: int = 32768  # int16 addressable rows per gather table half
    call_chunks: int = 16  # 128-idx chunks per dma_gather call (<=32)
    n_swdge_queues: int = 4  # parallel SWDGE desc-gen queues (1..4)
    neg_pad: bool = False  # pad gather idx lists with -1 (skipped) vs 0
    ag_splits: int = 2  # split the per-layer AllGather into this many pieces
    balance_iters: int = 1  # lo/hi-aware node assignment refinement passes


def _plan(cfg: GCNConfig, edge_index: np.ndarray):
    """Host graph prep: node->(core,block,slot) assignment, per-block sorted
    edge lists split by table half, padding, and all static counts."""
    import heapq

    N, C = cfg.n_nodes, cfg.n_cores
    nloc = N // C
    nb = (nloc + 127) // 128
    nlocp = nb * 128
    ntab = nlocp * C
    nblocks = C * nb

    src = np.asarray(edge_index[0], dtype=np.int64)
    dst = np.asarray(edge_index[1], dtype=np.int64)
    deg = np.bincount(dst, minlength=N).astype(np.int64) + 1  # + self loop
    dis = (1.0 / np.sqrt(deg.astype(np.float64))).astype(np.float32)

    # Load-balanced node->block assignment (LPT on message count = deg).
    def lpt_assign(key1, key2=None):
        """Greedy assignment minimizing per-block max of key1 (and key2 as a
        tiebreaker-ish combined potential). Returns node_row."""
        if key2 is None:
            key2 = np.zeros_like(key1)
        order_ = np.argsort(-(key1 + key2), kind="stable")
        rows = np.empty(N, dtype=np.int64)
        heap_ = [(0.0, b) for b in range(nblocks)]
        heapq.heapify(heap_)
        f1 = np.zeros(nblocks, dtype=np.int64)
        f2 = np.zeros(nblocks, dtype=np.int64)
        cnt_ = np.zeros(nblocks, dtype=np.int64)
        # scale so both coordinates contribute comparably to the potential
        s1 = 1.0 / max(1.0, key1.sum() / nblocks)
        s2 = 1.0 / max(1.0, key2.sum() / nblocks) if key2.any() else 0.0
        for n in order_:
            while True:
                f, b = heapq.heappop(heap_)
                if cnt_[b] < 128:
                    break
            rows[n] = b * 128 + cnt_[b]
            cnt_[b] += 1
            f1[b] += key1[n]
            f2[b] += key2[n]
            heapq.heappush(heap_, (f1[b] * s1 + f2[b] * s2, b))
        return rows

    # Table-row numbering for gather indices: with ag_splits=k the AllGather
    # runs as k block-aligned sub-collectives over shard slices, so the
    # physical table is slice-major: slice s holds every core's rows
    # [bnds[s], bnds[s+1]) back-to-back.  With k=2 the cut is placed exactly
    # at the int16 lo/hi table boundary so lo gathers can start after slice 0.
    k = cfg.ag_splits
    if k == 2 and cfg.half % (C * 128) == 0 and cfg.half // C < nlocp:
        bnds = [0, cfg.half // C, nlocp]
    else:
        bnds = [round(i * nb / k) * 128 for i in range(k + 1)]

    def to_tabrow(rows):
        # Slice-major, then core, then PARTITION-major within the core slice
        # (row = slot * nblocks_in_slice + block) so the AllGather bounce DMA
        # is one contiguous descriptor per SBUF partition.
        cc_ = rows // nlocp
        jj_ = rows % nlocp
        blk_ = jj_ // 128
        slot_ = jj_ % 128
        tr_ = np.empty_like(rows)
        off_ = 0
        for s in range(k):
            lo_b, hi_b = bnds[s], bnds[s + 1]
            sz = hi_b - lo_b
            nbs = sz // 128
            m = (jj_ >= lo_b) & (jj_ < hi_b)
            tr_[m] = (
                off_ + cc_[m] * sz + slot_[m] * nbs + (blk_[m] - lo_b // 128)
            )
            off_ += C * sz
        return tr_

    node_row = lpt_assign(deg)
    for _ in range(cfg.balance_iters):
        # lo/hi-aware refinement: classify each edge by its src's current
        # table half, re-balance blocks on (lo_in, hi_in) jointly.
        e_lo = to_tabrow(node_row)[src] < cfg.half
        lo_in = np.bincount(dst[e_lo], minlength=N)
        hi_in = np.bincount(dst[~e_lo], minlength=N)
        node_row = lpt_assign(lo_in, hi_in)

    tabrow = to_tabrow(node_row)

    # Per-block edge lists (excluding self loops; those are the SBUF chunk).
    srow = tabrow[src]
    drow = node_row[dst]
    eblk = drow // 128
    eslot = drow % 128
    o = np.lexsort((srow, eblk))
    srow_s, eblk_s, eslot_s = srow[o], eblk[o], eslot[o]
    starts = np.searchsorted(eblk_s, np.arange(nblocks + 1))

    lo_counts = np.empty(nblocks, dtype=np.int64)
    hi_counts = np.empty(nblocks, dtype=np.int64)
    for b in range(nblocks):
        s, e = starts[b], starts[b + 1]
        p = np.searchsorted(srow_s[s:e], cfg.half)
        lo_counts[b] = p
        hi_counts[b] = e - s - p
    nch_lo = int(max(1, math.ceil(lo_counts.max() / 128)))
    nch_hi = int(math.ceil(hi_counts.max() / 128)) if ntab > cfg.half else 0
    if ntab > cfg.half:
        nch_hi = max(1, nch_hi)
    nch_e = nch_lo + nch_hi

    # dis by node-row and by table-row (pads -> 0).
    dis_row = np.zeros(ntab, dtype=np.float32)
    dis_row[node_row] = dis
    dis_tabrow = np.zeros(ntab, dtype=np.float32)
    dis_tabrow[tabrow] = dis

    # Per-core packed idx (int16, 16-wrap replicated x8) and precomputed
    # selector matrices S (norm = dis[src]*dis[dst] folded in), organized as
    # two global chunk streams (lo chunks of all blocks in block order, then
    # hi chunks).  Gather calls cut every `call_chunks` chunks within a
    # stream, crossing block boundaries.  S[p, ch*128 + s] = norm of the
    # p-th message of chunk ch if its dst slot == s, else 0.
    n_lo_chunks = nb * nch_lo
    n_hi_chunks = nb * nch_hi
    nsegc = n_lo_chunks + n_hi_chunks
    t16 = nsegc * 8  # int16 columns per core
    idx_all = np.zeros((C, 128, t16), dtype=np.int16)
    S_all = np.zeros((C, 128, nsegc * 128), dtype=BF16)
    diag_all = np.zeros((C, 128, nb * 128), dtype=BF16)

    def pack_idx(vals, n_slots):
        a = np.zeros(n_slots, dtype=np.int16)
        a[: len(vals)] = vals
        return a.reshape(n_slots // 16, 16).T  # [16, n16]

    for c in range(C):
        dloc = dis_row[c * nlocp : (c + 1) * nlocp]
        for j in range(nb):
            diag_all[c, np.arange(128), j * 128 + np.arange(128)] = (
                dloc[j * 128 : (j + 1) * 128] ** 2
            ).astype(BF16)
            b = c * nb + j
            s, e = starts[b], starts[b + 1]
            p = lo_counts[b]
            lo_rows = srow_s[s : s + p]
            hi_rows = srow_s[s + p : e] - cfg.half
            lo_slot = eslot_s[s : s + p]
            hi_slot = eslot_s[s + p : e]
            for base_ch, nch, rows, slots, roff in (
                (j * nch_lo, nch_lo, lo_rows, lo_slot, 0),
                (n_lo_chunks + j * nch_hi, nch_hi, hi_rows, hi_slot, cfg.half),
            ):
                if nch == 0:
                    continue
                w16 = pack_idx(rows.astype(np.int16), nch * 128)
                idx_all[c, :, base_ch * 8 : (base_ch + nch) * 8] = np.tile(
                    w16, (8, 1)
                )
                m = len(rows)
                if m:
                    norm = (
                        dis_tabrow[rows + roff] * dloc[j * 128 + slots]
                    ).astype(BF16)
                    pos = np.arange(m) % 128
                    cols = (base_ch + np.arange(m) // 128) * 128 + slots
                    S_all[c, pos, cols] = norm
    return dict(
        nloc=nloc,
        nb=nb,
        nlocp=nlocp,
        ntab=ntab,
        nch_lo=nch_lo,
        nch_hi=nch_hi,
        nch_e=nch_e,
        n_lo_chunks=n_lo_chunks,
        n_hi_chunks=n_hi_chunks,
        t16=t16,
        nsegc=nsegc,
        node_row=node_row,
        ag_bnds=bnds,
        dis_row=dis_row,
        idx_all=idx_all,
        S_all=S_all,
        diag_all=diag_all,
    )


def _build(cfg: GCNConfig, plan):
    """Build the SPMD Bass program (identical across cores; per-core data
    arrives via ExternalInputs)."""
    import concourse.bacc as bacc
    import concourse.tile as tile
    from concourse import mybir

    dt = mybir.dt
    F, L, C = cfg.feat, cfg.n_layers, cfg.n_cores
    nb, nlocp, ntab = plan["nb"], plan["nlocp"], plan["ntab"]
    nloc = plan["nloc"]
    nch_lo, nch_hi, nch_e = plan["nch_lo"], plan["nch_hi"], plan["nch_e"]
    t16, nsegc = plan["t16"], plan["nsegc"]
    half = cfg.half

    n_lo_chunks, n_hi_chunks = plan["n_lo_chunks"], plan["n_hi_chunks"]
    W = cfg.call_chunks
    n_lo_calls = math.ceil(n_lo_chunks / W)
    n_hi_calls = math.ceil(n_hi_chunks / W) if n_hi_chunks else 0

    nc = bacc.Bacc(
        "TRN2",
        target_bir_lowering=False,
        debug=False,
        num_devices=C,
        num_swdge_queues=cfg.n_swdge_queues,
    )
    RG = [list(range(C))]

    xT_d = nc.dram_tensor("xT", [F, nlocp], dt.bfloat16, kind="ExternalInput")
    W_d = nc.dram_tensor("Wb", [L, F, F], dt.bfloat16, kind="ExternalInput")
    idx_d = nc.dram_tensor("idx", [128, t16], dt.int16, kind="ExternalInput")
    S_d = nc.dram_tensor("Stab", [128, nsegc * 128], dt.bfloat16,
                         kind="ExternalInput")
    diag_d = nc.dram_tensor("diag", [128, nb * 128], dt.bfloat16,
                            kind="ExternalInput")
    bcol_d = nc.dram_tensor("bcol", [128, L], dt.float32, kind="ExternalInput")
    linw_d = nc.dram_tensor("linw", [F, 1], dt.bfloat16, kind="ExternalInput")
    linb_d = nc.dram_tensor("linb", [1, 1], dt.float32, kind="ExternalInput")
    out_d = nc.dram_tensor("out", [1, nlocp], dt.float32, kind="ExternalOutput")

    bounces = [nc.dram_tensor(f"bounce{i}", [nlocp, F], dt.bfloat16) for i in range(2)]
    tables = [
        nc.dram_tensor(f"table{i}", [ntab, F], dt.bfloat16, addr_space="Shared")
        for i in range(2)
    ]

    with tile.TileContext(nc) as tc:
        with (
            tc.tile_pool(name="const", bufs=1) as const,
            tc.tile_pool(name="gplo", bufs=8) as gplo,
            tc.tile_pool(name="gphi", bufs=4) as gphi,
            tc.tile_pool(name="splo", bufs=8) as splo,
            tc.tile_pool(name="sphi", bufs=4) as sphi,
            tc.tile_pool(name="psA", bufs=3, space="PSUM") as psA,
            tc.tile_pool(name="psO", bufs=4, space="PSUM") as psO,
            tc.tile_pool(name="psL", bufs=1, space="PSUM") as psL,
        ):
            # ---- persistent tiles + one-time loads
            h0 = const.tile([F, nlocp], dt.bfloat16, tag="h0")
            h1 = const.tile([F, nlocp], dt.bfloat16, tag="h1")
            zbufs = [
                const.tile([128, nb * F], dt.bfloat16, tag=f"zbuf{i}", name=f"zbuf{i}")
                for i in range(2)
            ]
            W_sb = const.tile([F, L * F], dt.bfloat16, tag="W")
            idx_sb = const.tile([128, t16], dt.int16, tag="idx")
            diag = const.tile([128, nb * 128], dt.bfloat16, tag="diag")
            bcol = const.tile([128, L], dt.float32, tag="bcol")
            linw = const.tile([F, 1], dt.bfloat16, tag="linw")
            linb = const.tile([1, 1], dt.float32, tag="linb")
            orow = const.tile([1, nlocp], dt.float32, tag="orow")

            nc.sync.dma_start(out=h0[:], in_=xT_d[:])
            nc.sync.dma_start(
                out=W_sb[:].rearrange("p (l f) -> p l f", f=F),
                in_=W_d[:].rearrange("l p f -> p l f"),
            )
            nc.sync.dma_start(out=idx_sb[:], in_=idx_d[:])
            nc.sync.dma_start(out=diag[:], in_=diag_d[:])
            nc.sync.dma_start(out=bcol[:], in_=bcol_d[:])
            nc.sync.dma_start(out=linw[:], in_=linw_d[:])
            nc.sync.dma_start(out=linb[:], in_=linb_d[:])

            hs = [h0, h1]
            gq = [0]  # global gather-call counter for queue round-robin
            bnds = plan["ag_bnds"]
            ag_toff = []
            toff = 0
            for s in range(cfg.ag_splits):
                ag_toff.append(toff)
                toff += C * (bnds[s + 1] - bnds[s])

            def stage_a(l, j):
                jsl = slice(j * 128, (j + 1) * 128)
                zp = psA.tile([128, F], dt.float32, tag="zp")
                nc.tensor.matmul(
                    out=zp[:],
                    lhsT=hs[l % 2][:, jsl],
                    rhs=W_sb[:, l * F : (l + 1) * F],
                    start=True,
                    stop=True,
                )
                nc.scalar.activation(
                    out=zbufs[l % 2][:, jsl],
                    in_=zp[:],
                    func=mybir.ActivationFunctionType.Identity,
                )

            def emit_ag(l, s):
                zbuf = zbufs[l % 2]
                bounce = bounces[l % 2]
                table = tables[l % 2]
                lo_b, hi_b = bnds[s], bnds[s + 1]
                sz = hi_b - lo_b
                nc.sync.dma_start(
                    out=bounce[lo_b:hi_b, :].rearrange(
                        "(p b) f -> p (b f)", p=128
                    ),
                    in_=zbuf[:, lo_b * F // 128 : hi_b * F // 128],
                )
                nc.gpsimd.collective_compute(
                    "AllGather",
                    mybir.AluOpType.bypass,
                    replica_groups=RG,
                    ins=[bounce[lo_b:hi_b, :]],
                    outs=[table[ag_toff[s] : ag_toff[s] + C * sz, :]],
                )

            # prologue: stage A + AllGather for layer 0
            for j in range(nb):
                stage_a(0, j)
                for s in range(cfg.ag_splits):
                    if j == bnds[s + 1] // 128 - 1:
                        emit_ag(0, s)

            for l in range(L):
                zbuf = zbufs[l % 2]
                table = tables[l % 2]
                h_out = hs[(l + 1) % 2]
                # ---- stage B (layer l) interleaved with stage A/AG (l+1)
                lo_tiles = [None] * n_lo_calls
                lo_S = [None] * n_lo_calls
                hi_tiles = [None] * n_hi_calls
                hi_S = [None] * n_hi_calls

                def emit_call(k, base_ch, n_ch, tiles, Ss, tab, gp, sp):
                    w = min(W, n_ch - k * W)
                    ch0 = base_ch + k * W
                    g = gp.tile([128, W, F], dt.bfloat16, tag="g")
                    nc.gpsimd.dma_gather(
                        g[:, :w, :],
                        tab,
                        idx_sb[:, ch0 * 8 : (ch0 + w) * 8],
                        w * 128,
                        w * 128,
                        F,
                        elem_step=F,
                        single_packet=False,
                        queue_num=gq[0] % cfg.n_swdge_queues,
                    )
                    gq[0] += 1
                    S = sp.tile([128, W * 128], dt.bfloat16, tag="S")
                    nc.scalar.dma_start(
                        out=S[:, : w * 128],
                        in_=S_d[:, ch0 * 128 : (ch0 + w) * 128],
                    )
                    tiles[k] = g
                    Ss[k] = S

                tab_lo = table[0:half, :] if nch_hi else table[:, :]
                tab_hi = table[half:ntab, :]
                next_lo = [0]
                next_hi = [0]

                def need(j):
                    # emit gather calls covering block j's chunks
                    while next_lo[0] * W < min((j + 1) * nch_lo, n_lo_chunks):
                        emit_call(next_lo[0], 0, n_lo_chunks,
                                  lo_tiles, lo_S, tab_lo, gplo, splo)
                        next_lo[0] += 1
                    while next_hi[0] * W < min((j + 1) * nch_hi, n_hi_chunks):
                        emit_call(next_hi[0], n_lo_chunks, n_hi_chunks,
                                  hi_tiles, hi_S, tab_hi, gphi, sphi)
                        next_hi[0] += 1

                for j in range(nb):
                    jsl = slice(j * 128, (j + 1) * 128)
                    need(min(j + 1, nb - 1))
                    O = psO.tile([F, 128], dt.float32, tag="O")
                    nc.tensor.matmul(
                        out=O[:], lhsT=zbuf[:, jsl], rhs=diag[:, jsl],
                        start=True, stop=False,
                    )
                    chunks = [
                        (lo_tiles, lo_S, j * nch_lo + c) for c in range(nch_lo)
                    ] + [
                        (hi_tiles, hi_S, j * nch_hi + c) for c in range(nch_hi)
                    ]
                    for i, (tiles, Ss, ch) in enumerate(chunks):
                        k, off = ch // W, ch % W
                        nc.tensor.matmul(
                            out=O[:],
                            lhsT=tiles[k][:, off, :],
                            rhs=Ss[k][:, off * 128 : (off + 1) * 128],
                            start=False,
                            stop=(i == len(chunks) - 1),
                        )
                    nc.scalar.activation(
                        out=h_out[:, jsl],
                        in_=O[:],
                        func=mybir.ActivationFunctionType.Relu,
                        bias=bcol[:, l : l + 1],
                        scale=1.0,
                    )
                    if l + 1 < L:
                        stage_a(l + 1, j)
                        for s in range(cfg.ag_splits):
                            if j == bnds[s + 1] // 128 - 1:
                                emit_ag(l + 1, s)
            # ---- final linear readout
            h_fin = hs[L % 2]
            for k in range(0, nlocp, 512):
                w = min(512, nlocp - k)
                op = psL.tile([1, 512], dt.float32, tag="op")
                nc.tensor.matmul(
                    out=op[:, :w], lhsT=linw[:], rhs=h_fin[:, k : k + w],
                    start=True, stop=True,
                )
                nc.scalar.activation(
                    out=orow[:, k : k + w],
                    in_=op[:, :w],
                    func=mybir.ActivationFunctionType.Identity,
                    bias=linb[:],
                    scale=1.0,
                )
            nc.sync.dma_start(out=out_d[:], in_=orow[:])
    nc.compile()
    return nc


def _make_inputs(cfg: GCNConfig, plan, x, Ws, bs, lin_w, lin_b):
    """Per-core in_maps from full inputs + plan."""
    C, F, L = cfg.n_cores, cfg.feat, cfg.n_layers
    nlocp, nb = plan["nlocp"], plan["nb"]
    node_row = plan["node_row"]
    dis_row = plan["dis_row"]

    x = np.asarray(x, dtype=np.float32)
    Ws = np.asarray(Ws, dtype=np.float32)
    bs = np.asarray(bs, dtype=np.float32)
    lin_w = np.asarray(lin_w, dtype=np.float32)
    lin_b = np.asarray(lin_b, dtype=np.float32)

    xrow = np.zeros((C * nlocp, F), dtype=np.float32)
    xrow[node_row] = x
    Wb = Ws.astype(BF16)
    bcol = bs.T.astype(np.float32).copy()  # [F, L]
    linw = lin_w.reshape(F, 1).astype(BF16)
    linb = lin_b.reshape(1, 1).astype(np.float32)

    in_maps = []
    for c in range(C):
        rows = slice(c * nlocp, (c + 1) * nlocp)
        in_maps.append(
            {
                "xT": np.ascontiguousarray(xrow[rows].T).astype(BF16),
                "Wb": Wb,
                "idx": np.ascontiguousarray(plan["idx_all"][c]),
                "Stab": plan["S_all"][c],
                "diag": plan["diag_all"][c],
                "bcol": bcol,
                "linw": linw,
                "linb": linb,
            }
        )
    return in_maps


def _reassemble(cfg: GCNConfig, plan, outs):
    nlocp = plan["nlocp"]
    node_row = plan["node_row"]
    full = np.zeros(cfg.n_cores * nlocp, dtype=np.float32)
    for c, o in enumerate(outs):
        full[c * nlocp : (c + 1) * nlocp] = o["out"].reshape(-1)
    return full[node_row]


def kernel(**inputs) -> np.ndarray:
    cfg = GCNConfig()
    return _kernel_impl(cfg, inputs, mode=os.environ.get("GCN_MODE", "hw"))


def _kernel_impl(cfg: GCNConfig, inputs, mode="hw", trace=False):
    x = np.asarray(inputs["x"])
    edge_index = np.asarray(inputs["edge_index"])
    plan = _plan(cfg, edge_index)
    nc = _build(cfg, plan)
    in_maps = _make_inputs(
        cfg, plan, x, inputs["Ws"], inputs["bs"], inputs["lin_w"], inputs["lin_b"]
    )
    if mode == "sim":
        from concourse import bass_interp

        sim = bass_interp.MultiCoreSim(nc, cfg.n_cores)
        for c in range(cfg.n_cores):
            for k, v in in_maps[c].items():
                sim.cores[c].tensor(k)[:] = v
        sim.simulate()
        outs = [
            {"out": np.asarray(sim.cores[c].mem_tensor("out"))}
            for c in range(cfg.n_cores)
        ]
        result = _reassemble(cfg, plan, outs)
        return result.astype(np.float32)
    else:
        from concourse.bass_utils import run_bass_kernel_spmd

        res = run_bass_kernel_spmd(
            nc, in_maps, core_ids=list(range(cfg.n_cores)), trace=trace
        )
        out = _reassemble(cfg, plan, res.results)
        if trace:
            return out.astype(np.float32), res
        return out.astype(np.float32)


if __name__ == "__main__":
    pass



# revision 37
# speedup vs baseline: 1.0089x; 1.0089x over previous
"""GCN (7-layer, PyG-style symmetric-normalized message passing) on 8 TRN2
NeuronCores via Bass/Tile.

Strategy (graph/data parallel over nodes):
  - Nodes are assigned to 8 cores x 49 blocks of 128 slots each (load-balanced
    by in-degree so per-block message counts are nearly equal).
  - Per layer, per core:
      stage A: Z'' = dis * (h @ W) for the core's 6272 node slots (PE matmul
               per 128-node block + per-partition scale), node-major in SBUF.
      AllGather: bf16 Z'' shards -> full 50176-row table in local HBM.
      stage B: per dst block, gather Z''[src] rows for the block's edges
               (SWDGE dma_gather, int16 indices, table split in two halves to
               fit int16 range), build one-hot selector S on DVE
               (S[msg, slot] = (iota == segid)), and segment-sum on PE:
               O[feat, slot] += M_chunk.T @ S_chunk, with the self-loop chunk
               done as zbuf_block.T @ I directly from SBUF.
      epilogue: h' = relu(O * dis[dst] + b)  (DVE mult + ACT relu w/ bias).
  - Final: out = lin_w.T @ h7 + lin_b, one row per core, host reassembles.

All index/normalization prep is host-side numpy (graph routing), baked into
per-core input tensors; the float pipeline runs on device.
"""

import math
import os
import sys
from dataclasses import dataclass

import numpy as np

sys.path.insert(0, "/opt/trn_rl_repo")

import ml_dtypes  # noqa: E402

BF16 = ml_dtypes.bfloat16


@dataclass
class GCNConfig:
    n_nodes: int = 50000
    n_edges: int = 600000
    feat: int = 128
    n_layers: int = 7
    n_cores: int = 8
    half: int = 32768  # int16 addressable rows per gather table half
    call_chunks: int = 16  # 128-idx chunks per dma_gather call (<=32)
    n_swdge_queues: int = 4  # parallel SWDGE desc-gen queues (1..4)
    neg_pad: bool = False  # pad gather idx lists with -1 (skipped) vs 0
    ag_splits: int = 2  # split the per-layer AllGather into this many pieces
    balance_iters: int = 1  # lo/hi-aware node assignment refinement passes


def _plan(cfg: GCNConfig, edge_index: np.ndarray):
    """Host graph prep: node->(core,block,slot) assignment, per-block sorted
    edge lists split by table half, padding, and all static counts."""
    import heapq

    N, C = cfg.n_nodes, cfg.n_cores
    nloc = N // C
    nb = (nloc + 127) // 128
    nlocp = nb * 128
    ntab = nlocp * C
    nblocks = C * nb

    src = np.asarray(edge_index[0], dtype=np.int64)
    dst = np.asarray(edge_index[1], dtype=np.int64)
    deg = np.bincount(dst, minlength=N).astype(np.int64) + 1  # + self loop
    dis = (1.0 / np.sqrt(deg.astype(np.float64))).astype(np.float32)

    # Load-balanced node->block assignment (LPT on message count = deg).
    def lpt_assign(key1, key2=None):
        """Greedy assignment minimizing per-block max of key1 (and key2 as a
        tiebreaker-ish combined potential). Returns node_row."""
        if key2 is None:
            key2 = np.zeros_like(key1)
        order_ = np.argsort(-(key1 + key2), kind="stable")
        rows = np.empty(N, dtype=np.int64)
        heap_ = [(0.0, b) for b in range(nblocks)]
        heapq.heapify(heap_)
        f1 = np.zeros(nblocks, dtype=np.int64)
        f2 = np.zeros(nblocks, dtype=np.int64)
        cnt_ = np.zeros(nblocks, dtype=np.int64)
        # scale so both coordinates contribute comparably to the potential
        s1 = 1.0 / max(1.0, key1.sum() / nblocks)
        s2 = 1.0 / max(1.0, key2.sum() / nblocks) if key2.any() else 0.0
        for n in order_:
            while True:
                f, b = heapq.heappop(heap_)
                if cnt_[b] < 128:
                    break
            rows[n] = b * 128 + cnt_[b]
            cnt_[b] += 1
            f1[b] += key1[n]
            f2[b] += key2[n]
            heapq.heappush(heap_, (f1[b] * s1 + f2[b] * s2, b))
        return rows

    # Table-row numbering for gather indices: with ag_splits=k the AllGather
    # runs as k block-aligned sub-collectives over shard slices, so the
    # physical table is slice-major: slice s holds every core's rows
    # [bnds[s], bnds[s+1]) back-to-back.  With k=2 the cut is placed exactly
    # at the int16 lo/hi table boundary so lo gathers can start after slice 0.
    k = cfg.ag_splits
    if k == 2 and cfg.half % (C * 128) == 0 and cfg.half // C < nlocp:
        bnds = [0, cfg.half // C, nlocp]
    else:
        bnds = [round(i * nb / k) * 128 for i in range(k + 1)]

    def to_tabrow(rows):
        # Slice-major, then core, then PARTITION-major within the core slice
        # (row = slot * nblocks_in_slice + block) so the AllGather bounce DMA
        # is one contiguous descriptor per SBUF partition.
        cc_ = rows // nlocp
        jj_ = rows % nlocp
        blk_ = jj_ // 128
        slot_ = jj_ % 128
        tr_ = np.empty_like(rows)
        off_ = 0
        for s in range(k):
            lo_b, hi_b = bnds[s], bnds[s + 1]
            sz = hi_b - lo_b
            nbs = sz // 128
            m = (jj_ >= lo_b) & (jj_ < hi_b)
            tr_[m] = (
                off_ + cc_[m] * sz + slot_[m] * nbs + (blk_[m] - lo_b // 128)
            )
            off_ += C * sz
        return tr_

    node_row = lpt_assign(deg)
    for _ in range(cfg.balance_iters):
        # lo/hi-aware refinement: classify each edge by its src's current
        # table half, re-balance blocks on (lo_in, hi_in) jointly.
        e_lo = to_tabrow(node_row)[src] < cfg.half
        lo_in = np.bincount(dst[e_lo], minlength=N)
        hi_in = np.bincount(dst[~e_lo], minlength=N)
        node_row = lpt_assign(lo_in, hi_in)

    tabrow = to_tabrow(node_row)

    # Per-block edge lists (excluding self loops; those are the SBUF chunk).
    srow = tabrow[src]
    drow = node_row[dst]
    eblk = drow // 128
    eslot = drow % 128
    o = np.lexsort((srow, eblk))
    srow_s, eblk_s, eslot_s = srow[o], eblk[o], eslot[o]
    starts = np.searchsorted(eblk_s, np.arange(nblocks + 1))

    lo_counts = np.empty(nblocks, dtype=np.int64)
    hi_counts = np.empty(nblocks, dtype=np.int64)
    for b in range(nblocks):
        s, e = starts[b], starts[b + 1]
        p = np.searchsorted(srow_s[s:e], cfg.half)
        lo_counts[b] = p
        hi_counts[b] = e - s - p
    nch_lo = int(max(1, math.ceil(lo_counts.max() / 128)))
    nch_hi = int(math.ceil(hi_counts.max() / 128)) if ntab > cfg.half else 0
    if ntab > cfg.half:
        nch_hi = max(1, nch_hi)
    nch_e = nch_lo + nch_hi

    # dis by node-row and by table-row (pads -> 0).
    dis_row = np.zeros(ntab, dtype=np.float32)
    dis_row[node_row] = dis
    dis_tabrow = np.zeros(ntab, dtype=np.float32)
    dis_tabrow[tabrow] = dis

    # Per-core packed idx (int16, 16-wrap replicated x8) and precomputed
    # selector matrices S (norm = dis[src]*dis[dst] folded in), organized as
    # two global chunk streams (lo chunks of all blocks in block order, then
    # hi chunks).  Gather calls cut every `call_chunks` chunks within a
    # stream, crossing block boundaries.  S[p, ch*128 + s] = norm of the
    # p-th message of chunk ch if its dst slot == s, else 0.
    n_lo_chunks = nb * nch_lo
    n_hi_chunks = nb * nch_hi
    nsegc = n_lo_chunks + n_hi_chunks
    t16 = nsegc * 8  # int16 columns per core
    idx_all = np.zeros((C, 128, t16), dtype=np.int16)
    S_all = np.zeros((C, 128, nsegc * 128), dtype=BF16)
    diag_all = np.zeros((C, 128, nb * 128), dtype=BF16)

    def pack_idx(vals, n_slots):
        a = np.zeros(n_slots, dtype=np.int16)
        a[: len(vals)] = vals
        return a.reshape(n_slots // 16, 16).T  # [16, n16]

    for c in range(C):
        dloc = dis_row[c * nlocp : (c + 1) * nlocp]
        for j in range(nb):
            diag_all[c, np.arange(128), j * 128 + np.arange(128)] = (
                dloc[j * 128 : (j + 1) * 128] ** 2
            ).astype(BF16)
            b = c * nb + j
            s, e = starts[b], starts[b + 1]
            p = lo_counts[b]
            lo_rows = srow_s[s : s + p]
            hi_rows = srow_s[s + p : e] - cfg.half
            lo_slot = eslot_s[s : s + p]
            hi_slot = eslot_s[s + p : e]
            for base_ch, nch, rows, slots, roff in (
                (j * nch_lo, nch_lo, lo_rows, lo_slot, 0),
                (n_lo_chunks + j * nch_hi, nch_hi, hi_rows, hi_slot, cfg.half),
            ):
                if nch == 0:
                    continue
                w16 = pack_idx(rows.astype(np.int16), nch * 128)
                idx_all[c, :, base_ch * 8 : (base_ch + nch) * 8] = np.tile(
                    w16, (8, 1)
                )
                m = len(rows)
                if m:
                    norm = (
                        dis_tabrow[rows + roff] * dloc[j * 128 + slots]
                    ).astype(BF16)
                    pos = np.arange(m) % 128
                    cols = (base_ch + np.arange(m) // 128) * 128 + slots
                    S_all[c, pos, cols] = norm
    return dict(
        nloc=nloc,
        nb=nb,
        nlocp=nlocp,
        ntab=ntab,
        nch_lo=nch_lo,
        nch_hi=nch_hi,
        nch_e=nch_e,
        n_lo_chunks=n_lo_chunks,
        n_hi_chunks=n_hi_chunks,
        t16=t16,
        nsegc=nsegc,
        node_row=node_row,
        ag_bnds=bnds,
        dis_row=dis_row,
        idx_all=idx_all,
        S_all=S_all,
        diag_all=diag_all,
    )


def _build(cfg: GCNConfig, plan):
    """Build the SPMD Bass program (identical across cores; per-core data
    arrives via ExternalInputs)."""
    import concourse.bacc as bacc
    import concourse.tile as tile
    from concourse import mybir

    dt = mybir.dt
    F, L, C = cfg.feat, cfg.n_layers, cfg.n_cores
    nb, nlocp, ntab = plan["nb"], plan["nlocp"], plan["ntab"]
    nloc = plan["nloc"]
    nch_lo, nch_hi, nch_e = plan["nch_lo"], plan["nch_hi"], plan["nch_e"]
    t16, nsegc = plan["t16"], plan["nsegc"]
    half = cfg.half

    n_lo_chunks, n_hi_chunks = plan["n_lo_chunks"], plan["n_hi_chunks"]
    W = cfg.call_chunks
    n_lo_calls = math.ceil(n_lo_chunks / W)
    n_hi_calls = math.ceil(n_hi_chunks / W) if n_hi_chunks else 0

    nc = bacc.Bacc(
        "TRN2",
        target_bir_lowering=False,
        debug=False,
        num_devices=C,
        num_swdge_queues=cfg.n_swdge_queues,
    )
    RG = [list(range(C))]

    xT_d = nc.dram_tensor("xT", [F, nlocp], dt.bfloat16, kind="ExternalInput")
    W_d = nc.dram_tensor("Wb", [L, F, F], dt.bfloat16, kind="ExternalInput")
    idx_d = nc.dram_tensor("idx", [128, t16], dt.int16, kind="ExternalInput")
    S_d = nc.dram_tensor("Stab", [128, nsegc * 128], dt.bfloat16,
                         kind="ExternalInput")
    diag_d = nc.dram_tensor("diag", [128, nb * 128], dt.bfloat16,
                            kind="ExternalInput")
    bcol_d = nc.dram_tensor("bcol", [128, L], dt.float32, kind="ExternalInput")
    linw_d = nc.dram_tensor("linw", [F, 1], dt.bfloat16, kind="ExternalInput")
    linb_d = nc.dram_tensor("linb", [1, 1], dt.float32, kind="ExternalInput")
    out_d = nc.dram_tensor("out", [1, nlocp], dt.float32, kind="ExternalOutput")

    bounces = [nc.dram_tensor(f"bounce{i}", [nlocp, F], dt.bfloat16) for i in range(2)]
    tables = [
        nc.dram_tensor(f"table{i}", [ntab, F], dt.bfloat16, addr_space="Shared")
        for i in range(2)
    ]

    with tile.TileContext(nc) as tc:
        with (
            tc.tile_pool(name="const", bufs=1) as const,
            tc.tile_pool(name="gplo", bufs=10) as gplo,
            tc.tile_pool(name="gphi", bufs=5) as gphi,
            tc.tile_pool(name="splo", bufs=10) as splo,
            tc.tile_pool(name="sphi", bufs=5) as sphi,
            tc.tile_pool(name="opool", bufs=2) as opool,
            tc.tile_pool(name="psA", bufs=3, space="PSUM") as psA,
            tc.tile_pool(name="psO", bufs=4, space="PSUM") as psO,
            tc.tile_pool(name="psL", bufs=1, space="PSUM") as psL,
        ):
            # ---- persistent tiles + one-time loads
            h0 = const.tile([F, nlocp], dt.bfloat16, tag="h0")
            h1 = const.tile([F, nlocp], dt.bfloat16, tag="h1")
            zbufs = [
                const.tile([128, nb * F], dt.bfloat16, tag=f"zbuf{i}", name=f"zbuf{i}")
                for i in range(2)
            ]
            W_sb = const.tile([F, L * F], dt.bfloat16, tag="W")
            idx_sb = const.tile([128, t16], dt.int16, tag="idx")
            diag = const.tile([128, nb * 128], dt.bfloat16, tag="diag")
            bcol = const.tile([128, L], dt.float32, tag="bcol")
            linw = const.tile([F, 1], dt.bfloat16, tag="linw")
            linb = const.tile([1, 1], dt.float32, tag="linb")

            nc.sync.dma_start(out=h0[:], in_=xT_d[:])
            nc.sync.dma_start(
                out=W_sb[:].rearrange("p (l f) -> p l f", f=F),
                in_=W_d[:].rearrange("l p f -> p l f"),
            )
            nc.sync.dma_start(out=idx_sb[:], in_=idx_d[:])
            nc.sync.dma_start(out=diag[:], in_=diag_d[:])
            nc.sync.dma_start(out=bcol[:], in_=bcol_d[:])
            nc.sync.dma_start(out=linw[:], in_=linw_d[:])
            nc.sync.dma_start(out=linb[:], in_=linb_d[:])

            hs = [h0, h1]
            gq = [0]  # global gather-call counter for queue round-robin
            bnds = plan["ag_bnds"]
            ag_toff = []
            toff = 0
            for s in range(cfg.ag_splits):
                ag_toff.append(toff)
                toff += C * (bnds[s + 1] - bnds[s])

            def stage_a(l, j):
                jsl = slice(j * 128, (j + 1) * 128)
                zp = psA.tile([128, F], dt.float32, tag="zp")
                nc.tensor.matmul(
                    out=zp[:],
                    lhsT=hs[l % 2][:, jsl],
                    rhs=W_sb[:, l * F : (l + 1) * F],
                    start=True,
                    stop=True,
                )
                nc.scalar.activation(
                    out=zbufs[l % 2][:, jsl],
                    in_=zp[:],
                    func=mybir.ActivationFunctionType.Identity,
                )

            def emit_ag(l, s):
                zbuf = zbufs[l % 2]
                bounce = bounces[l % 2]
                table = tables[l % 2]
                lo_b, hi_b = bnds[s], bnds[s + 1]
                sz = hi_b - lo_b
                nc.scalar.dma_start(
                    out=bounce[lo_b:hi_b, :].rearrange(
                        "(p b) f -> p (b f)", p=128
                    ),
                    in_=zbuf[:, lo_b * F // 128 : hi_b * F // 128],
                )
                nc.gpsimd.collective_compute(
                    "AllGather",
                    mybir.AluOpType.bypass,
                    replica_groups=RG,
                    ins=[bounce[lo_b:hi_b, :]],
                    outs=[table[ag_toff[s] : ag_toff[s] + C * sz, :]],
                )

            # prologue: stage A + AllGather for layer 0
            for j in range(nb):
                stage_a(0, j)
                for s in range(cfg.ag_splits):
                    if j == bnds[s + 1] // 128 - 1:
                        emit_ag(0, s)

            for l in range(L):
                zbuf = zbufs[l % 2]
                table = tables[l % 2]
                h_out = hs[(l + 1) % 2]
                # ---- stage B (layer l) interleaved with stage A/AG (l+1)
                lo_tiles = [None] * n_lo_calls
                lo_S = [None] * n_lo_calls
                hi_tiles = [None] * n_hi_calls
                hi_S = [None] * n_hi_calls

                def emit_call(k, base_ch, n_ch, tiles, Ss, tab, gp, sp):
                    w = min(W, n_ch - k * W)
                    ch0 = base_ch + k * W
                    g = gp.tile([128, W, F], dt.bfloat16, tag="g")
                    nc.gpsimd.dma_gather(
                        g[:, :w, :],
                        tab,
                        idx_sb[:, ch0 * 8 : (ch0 + w) * 8],
                        w * 128,
                        w * 128,
                        F,
                        elem_step=F,
                        single_packet=False,
                        queue_num=gq[0] % cfg.n_swdge_queues,
                    )
                    gq[0] += 1
                    S = sp.tile([128, W * 128], dt.bfloat16, tag="S")
                    nc.sync.dma_start(
                        out=S[:, : w * 128],
                        in_=S_d[:, ch0 * 128 : (ch0 + w) * 128],
                    )
                    tiles[k] = g
                    Ss[k] = S

                tab_lo = table[0:half, :] if nch_hi else table[:, :]
                tab_hi = table[half:ntab, :]
                next_lo = [0]
                next_hi = [0]

                def need(j):
                    # emit gather calls covering block j's chunks; lo runs
                    # 2 blocks ahead, hi exactly on time (so early hi calls
                    # don't head-of-line block ready lo calls on the Pool
                    # queue while this layer's AG-hi slice finishes)
                    jlo = min(j + 2, nb - 1)
                    while next_lo[0] * W < min((jlo + 1) * nch_lo, n_lo_chunks):
                        emit_call(next_lo[0], 0, n_lo_chunks,
                                  lo_tiles, lo_S, tab_lo, gplo, splo)
                        next_lo[0] += 1
                    while next_hi[0] * W < min((j + 1) * nch_hi, n_hi_chunks):
                        emit_call(next_hi[0], n_lo_chunks, n_hi_chunks,
                                  hi_tiles, hi_S, tab_hi, gphi, sphi)
                        next_hi[0] += 1

                for j in range(nb):
                    jsl = slice(j * 128, (j + 1) * 128)
                    need(j)
                    O = psO.tile([F, 128], dt.float32, tag="O")
                    nc.tensor.matmul(
                        out=O[:], lhsT=zbuf[:, jsl], rhs=diag[:, jsl],
                        start=True, stop=False,
                    )
                    chunks = [
                        (lo_tiles, lo_S, j * nch_lo + c) for c in range(nch_lo)
                    ] + [
                        (hi_tiles, hi_S, j * nch_hi + c) for c in range(nch_hi)
                    ]
                    for i, (tiles, Ss, ch) in enumerate(chunks):
                        k, off = ch // W, ch % W
                        nc.tensor.matmul(
                            out=O[:],
                            lhsT=tiles[k][:, off, :],
                            rhs=Ss[k][:, off * 128 : (off + 1) * 128],
                            start=False,
                            stop=(i == len(chunks) - 1),
                        )
                    nc.scalar.activation(
                        out=h_out[:, jsl],
                        in_=O[:],
                        func=mybir.ActivationFunctionType.Relu,
                        bias=bcol[:, l : l + 1],
                        scale=1.0,
                    )
                    if l + 1 < L:
                        stage_a(l + 1, j)
                        for s in range(cfg.ag_splits):
                            if j == bnds[s + 1] // 128 - 1:
                                emit_ag(l + 1, s)
            # ---- final linear readout
            h_fin = hs[L % 2]
            for k in range(0, nlocp, 512):
                w = min(512, nlocp - k)
                op = psL.tile([1, 512], dt.float32, tag="op")
                nc.tensor.matmul(
                    out=op[:, :w], lhsT=linw[:], rhs=h_fin[:, k : k + w],
                    start=True, stop=True,
                )
                ot = opool.tile([1, 512], dt.float32, tag="ot")
                nc.scalar.activation(
                    out=ot[:, :w],
                    in_=op[:, :w],
                    func=mybir.ActivationFunctionType.Identity,
                    bias=linb[:],
                    scale=1.0,
                )
                nc.sync.dma_start(out=out_d[:, k : k + w], in_=ot[:, :w])
    nc.compile()
    return nc


def _make_inputs(cfg: GCNConfig, plan, x, Ws, bs, lin_w, lin_b):
    """Per-core in_maps from full inputs + plan."""
    C, F, L = cfg.n_cores, cfg.feat, cfg.n_layers
    nlocp, nb = plan["nlocp"], plan["nb"]
    node_row = plan["node_row"]
    dis_row = plan["dis_row"]

    x = np.asarray(x, dtype=np.float32)
    Ws = np.asarray(Ws, dtype=np.float32)
    bs = np.asarray(bs, dtype=np.float32)
    lin_w = np.asarray(lin_w, dtype=np.float32)
    lin_b = np.asarray(lin_b, dtype=np.float32)

    xrow = np.zeros((C * nlocp, F), dtype=np.float32)
    xrow[node_row] = x
    Wb = Ws.astype(BF16)
    bcol = bs.T.astype(np.float32).copy()  # [F, L]
    linw = lin_w.reshape(F, 1).astype(BF16)
    linb = lin_b.reshape(1, 1).astype(np.float32)

    in_maps = []
    for c in range(C):
        rows = slice(c * nlocp, (c + 1) * nlocp)
        in_maps.append(
            {
                "xT": np.ascontiguousarray(xrow[rows].T).astype(BF16),
                "Wb": Wb,
                "idx": np.ascontiguousarray(plan["idx_all"][c]),
                "Stab": plan["S_all"][c],
                "diag": plan["diag_all"][c],
                "bcol": bcol,
                "linw": linw,
                "linb": linb,
            }
        )
    return in_maps


def _reassemble(cfg: GCNConfig, plan, outs):
    nlocp = plan["nlocp"]
    node_row = plan["node_row"]
    full = np.zeros(cfg.n_cores * nlocp, dtype=np.float32)
    for c, o in enumerate(outs):
        full[c * nlocp : (c + 1) * nlocp] = o["out"].reshape(-1)
    return full[node_row]


def kernel(**inputs) -> np.ndarray:
    cfg = GCNConfig()
    return _kernel_impl(cfg, inputs, mode=os.environ.get("GCN_MODE", "hw"))


def _kernel_impl(cfg: GCNConfig, inputs, mode="hw", trace=False):
    x = np.asarray(inputs["x"])
    edge_index = np.asarray(inputs["edge_index"])
    plan = _plan(cfg, edge_index)
    nc = _build(cfg, plan)
    in_maps = _make_inputs(
        cfg, plan, x, inputs["Ws"], inputs["bs"], inputs["lin_w"], inputs["lin_b"]
    )
    if mode == "sim":
        from concourse import bass_interp

        sim = bass_interp.MultiCoreSim(nc, cfg.n_cores)
        for c in range(cfg.n_cores):
            for k, v in in_maps[c].items():
                sim.cores[c].tensor(k)[:] = v
        sim.simulate()
        outs = [
            {"out": np.asarray(sim.cores[c].mem_tensor("out"))}
            for c in range(cfg.n_cores)
        ]
        result = _reassemble(cfg, plan, outs)
        return result.astype(np.float32)
    else:
        from concourse.bass_utils import run_bass_kernel_spmd

        res = run_bass_kernel_spmd(
            nc, in_maps, core_ids=list(range(cfg.n_cores)), trace=trace
        )
        out = _reassemble(cfg, plan, res.results)
        if trace:
            return out.astype(np.float32), res
        return out.astype(np.float32)


if __name__ == "__main__":
    pass

